# revision 7
# baseline (speedup 1.0000x reference)
"""GNN message-passing (3x GraphConv + mean-pool + classifier) on 8 Trainium2 cores.

Strategy (self-contained, hardcoded for the nn_Classifier_74019466379909 shapes):
  - Nodes dst-sharded 8 ways (12500/core). Edges assigned to dst-owner core.
  - Per layer: T = (ns*h) @ W computed on own nodes, AllGather -> full Y table
    (fp16, node-major rows); aggregation gathers Y[src] rows via dma_gather
    (4 SWDGE queues, int16 page-local indices over 4x25000-row pages) and
    segment-sums them on the TensorEngine with on-device-built 0/1 one-hot
    matrices (DVE iota-compare). Norms are folded: norm_src into the next
    transform, norm_dst (+relu) into a single fused DVE op per node block.
  - Graph mean-pool via a host-built per-node (1/cnt) one-hot P stream on PE;
    tiny per-core partial logits are summed on the host (boundary graphs
    overlap two cores).
"""
import sys
import numpy as np

sys.path.insert(0, "/opt/trn_rl_repo")

import concourse.bass as bass  # noqa: E402
import concourse.bacc as bacc  # noqa: E402
import concourse.mybir as mybir  # noqa: E402
import concourse.tile as tile  # noqa: E402
from concourse.masks import make_identity  # noqa: E402
from concourse.bass_utils import run_bass_kernel_spmd  # noqa: E402

# problem constants
N_NODES = 100000
N_EDGES = 1600000
N_GRAPHS = 1000
IN_DIM, HID, N_CLS = 95, 128, 16

NCORES = 8
B = N_NODES // NCORES            # 12500 nodes per core
NBLK = (B + 127) // 128          # 98 blocks (last = 84 rows)
LASTN = B - 127 * (NBLK - 1) - (128 - 128)  # rows in last block
LASTN = B - (NBLK - 1) * 128     # 84
WINB = 4                         # blocks per window
NWIN = (NBLK + WINB - 1) // WINB  # 25 windows (last has 2 real blocks)
NPAGE = 4
PAGE = N_NODES // NPAGE          # 25000 rows per gather page
BUDGET = 5                       # chunks (of 128 idx) per (block x page) cell
CELL = BUDGET * 128              # 640 idx slots per cell
CALL = WINB * CELL               # 2560 idxs per dma_gather call
CCOL = CALL // 16                # 160 int16 cols per call in idx stream
NCALL = NWIN * NPAGE             # 100 calls per layer
CHPC = WINB * BUDGET             # 20 chunks per call
NCHUNK = NCALL * CHPC            # 2000 chunks per layer
GSPAN = 192                      # padded per-core graph span for pooling

F16 = mybir.dt.float16
F32 = mybir.dt.float32
I16 = mybir.dt.int16

_COMPILED = None  # (nc,) cache across calls in one process


def _host_prep(x, src, dst, graph_id, W1, b1, W2, b2, W3, b3, Wc, bc):
    """Build all per-core input streams. Index math only (+ dtype marshaling)."""
    src = np.asarray(src).astype(np.int64)
    dst = np.asarray(dst).astype(np.int64)
    graph_id = np.asarray(graph_id).astype(np.int64)
    x = np.asarray(x, dtype=np.float32)
    assert np.all(np.asarray(b1) == 0) and np.all(np.asarray(b2) == 0) and np.all(
        np.asarray(b3) == 0
    ), "kernel assumes zero conv biases (relu/scale folding)"

    deg_out = np.clip(np.bincount(src, minlength=N_NODES), 1, None).astype(np.float64)
    deg_in = np.clip(np.bincount(dst, minlength=N_NODES), 1, None).astype(np.float64)
    ns = (deg_out ** -0.5).astype(np.float32)
    nd = (deg_in ** -0.5).astype(np.float32)
    cnt = np.clip(np.bincount(graph_id, minlength=N_GRAPHS), 1, None).astype(np.float32)

    core_of = dst // B
    per_core = []
    for c in range(NCORES):
        m = core_of == c
        es = src[m]
        ed = dst[m] - c * B
        blk = ed >> 7
        page = es // PAGE
        lrow = (es - page * PAGE).astype(np.int64)
        slot = (ed & 127).astype(np.int64)

        idx_stream = np.zeros((NCALL, CALL), np.int64)  # page-local row per slot
        slot_stream = np.full((NCALL, CALL), -1.0, np.float32)

        # bucket edges by (window, page, block-in-window)
        order = np.lexsort((es, page, blk))
        es_o, blk_o, page_o, lrow_o, slot_o = (
            es[order], blk[order], page[order], lrow[order], slot[order])
        cell_key = blk_o * NPAGE + page_o
        starts = np.searchsorted(cell_key, np.arange(NBLK * NPAGE + 1))
        for b in range(NBLK):
            w, cw = divmod(b, WINB)
            for q in range(NPAGE):
                lo, hi = starts[b * NPAGE + q], starts[b * NPAGE + q + 1]
                n = hi - lo
                assert n <= CELL, f"cell overflow core {c} blk {b} page {q}: {n}"
                call_id = w * NPAGE + q
                base = cw * CELL
                idx_stream[call_id, base:base + n] = lrow_o[lo:hi]
                slot_stream[call_id, base:base + n] = slot_o[lo:hi]
                if n < CELL:  # mid-call pads: repeat a valid row (slot=-1)
                    fill = lrow_o[hi - 1] if n > 0 else 0
                    idx_stream[call_id, base + n:base + CELL] = fill

        # wrap idxs: flat position i -> [i%16, i//16], replicated to 128 partitions
        idx16 = idx_stream.reshape(NCALL, CCOL, 16).transpose(0, 2, 1)  # [call,16,160]
        idx128 = np.tile(idx16, (1, 8, 1)).transpose(1, 0, 2).reshape(128, NCALL * CCOL)
        # slot stream: chunk j covers positions [128j,128j+128); [128, NCHUNK]
        slot128 = slot_stream.reshape(NCALL * CHPC, 128).T.copy()

        # pooling stream
        gid_c = graph_id[c * B:(c + 1) * B]
        g0 = int(gid_c[0])
        span = int(gid_c[-1]) - g0 + 1
        assert span <= GSPAN, f"graph span {span} > {GSPAN}"
        pstream = np.zeros((128, NBLK * GSPAN), np.float32)
        for b in range(NBLK):
            nb = 128 if b < NBLK - 1 else LASTN
            rows = gid_c[b * 128:b * 128 + nb] - g0
            pstream[np.arange(nb), b * GSPAN + rows] = 1.0 / cnt[gid_c[b * 128:b * 128 + nb]]

        # per-node scalar streams [128, NBLK]
        def colify(v):
            out = np.zeros((128, NBLK), np.float32)
            flat = v[c * B:(c + 1) * B]
            out[:, :NBLK - 1] = flat[:(NBLK - 1) * 128].reshape(NBLK - 1, 128).T
            out[:LASTN, NBLK - 1] = flat[(NBLK - 1) * 128:]
            return out

        xT = np.ascontiguousarray(x[c * B:(c + 1) * B].T).astype(np.float16)  # [95,B]

        per_core.append({
            "xT": xT,
            "idxs": idx128.astype(np.int16),
            "slots": slot128.astype(np.float16),
            "sc1": colify(ns),
            "sc12": colify(ns * nd),
            "sc3": colify(nd),
            "pstr": pstream.astype(np.float16),
            "g0": g0, "span": span,
        })

    iota = np.broadcast_to(np.arange(128, dtype=np.float16), (128, 8, 128)).copy()
    iota = np.ascontiguousarray(np.broadcast_to(
        np.arange(128, dtype=np.float16)[None, None, :], (128, 8, 128)))
    shared = {
        "iota8": iota.reshape(128, 8 * 128),
        "W1f": np.asarray(W1, np.float32).astype(np.float16),
        "W2f": np.asarray(W2, np.float32).astype(np.float16),
        "W3f": np.asarray(W3, np.float32).astype(np.float16),
        "Wcf": np.asarray(Wc, np.float32).astype(np.float16),
    }
    return per_core, shared, cnt


def _build_nc():
    nc = bacc.Bacc("TRN2", target_bir_lowering=False, debug=False,
                   num_devices=NCORES, num_swdge_queues=4)
    xT = nc.dram_tensor("xT", [IN_DIM, B], F16, kind="ExternalInput")
    idxs = nc.dram_tensor("idxs", [128, NCALL * CCOL], I16, kind="ExternalInput")
    slots = nc.dram_tensor("slots", [128, NCHUNK], F16, kind="ExternalInput")
    iota8 = nc.dram_tensor("iota8", [128, 8 * 128], F16, kind="ExternalInput")
    W1f = nc.dram_tensor("W1f", [IN_DIM, HID], F16, kind="ExternalInput")
    W2f = nc.dram_tensor("W2f", [HID, HID], F16, kind="ExternalInput")
    W3f = nc.dram_tensor("W3f", [HID, HID], F16, kind="ExternalInput")
    Wcf = nc.dram_tensor("Wcf", [HID, N_CLS], F16, kind="ExternalInput")
    sc1 = nc.dram_tensor("sc1", [128, NBLK], F32, kind="ExternalInput")
    sc12 = nc.dram_tensor("sc12", [128, NBLK], F32, kind="ExternalInput")
    sc3 = nc.dram_tensor("sc3", [128, NBLK], F32, kind="ExternalInput")
    pstr = nc.dram_tensor("pstr", [128, NBLK * GSPAN], F16, kind="ExternalInput")
    zc = nc.dram_tensor("zc", [N_CLS, GSPAN], F32, kind="ExternalOutput")

    with tile.TileContext(nc) as tc:
        with (
            tc.tile_pool(name="res", bufs=1) as res,
            tc.tile_pool(name="dram", bufs=1, space="DRAM") as dram,
            tc.tile_pool(name="gew", bufs=8) as gew,
            tc.tile_pool(name="cw", bufs=8) as cw,
            tc.tile_pool(name="eps", bufs=4) as eps,
            tc.tile_pool(name="pwin", bufs=2) as pwin,
            tc.tile_pool(name="spsum", bufs=2, space="PSUM") as spsum,
            tc.tile_pool(name="tpsum", bufs=2, space="PSUM") as tpsum,
            tc.tile_pool(name="trpsum", bufs=2, space="PSUM") as trpsum,
            tc.tile_pool(name="gpsum", bufs=1, space="PSUM") as gpsum,
        ):
            # resident loads
            idx_t = res.tile([128, NCALL * CCOL], I16)
            slot_t = res.tile([128, NCHUNK], F16)
            iota_t = res.tile([128, 8, 128], F16)
            xT_t = res.tile([IN_DIM, B], F16)
            w1_t = res.tile([IN_DIM, HID], F16)
            w2_t = res.tile([HID, HID], F16)
            w3_t = res.tile([HID, HID], F16)
            wc_t = res.tile([HID, N_CLS], F16)
            sc1_t = res.tile([128, NBLK], F32)
            sc12_t = res.tile([128, NBLK], F32)
            sc3_t = res.tile([128, NBLK], F32)
            ident = res.tile([128, 128], F32)
            identh = res.tile([128, 128], F16)
            nc.sync.dma_start(idx_t[:], idxs[:])
            nc.sync.dma_start(slot_t[:], slots[:])
            nc.sync.dma_start(iota_t[:], iota8[:].rearrange("p (a b) -> p a b", a=8))
            nc.sync.dma_start(xT_t[:], xT[:])
            nc.sync.dma_start(w1_t[:], W1f[:])
            nc.sync.dma_start(w2_t[:], W2f[:])
            nc.sync.dma_start(w3_t[:], W3f[:])
            nc.sync.dma_start(wc_t[:], Wcf[:])
            nc.sync.dma_start(sc1_t[:], sc1[:])
            nc.sync.dma_start(sc12_t[:], sc12[:])
            nc.sync.dma_start(sc3_t[:], sc3[:])
            make_identity(nc, ident[:])
            nc.vector.tensor_copy(identh[:], ident[:])

            ybounce = [dram.tile([B, HID], F16, name=f"yb{i}", tag=f"yb{i}") for i in range(3)]
            yfull = [dram.tile([N_NODES, HID], F16, name=f"yf{i}", tag=f"yf{i}") for i in range(3)]

            # ---- phase T1: yb1 = (x @ W1) * ns (node-major fp16) ----
            for b in range(NBLK):
                nb = 128 if b < NBLK - 1 else LASTN
                tps = tpsum.tile([128, HID], F32)
                nc.tensor.matmul(tps[:nb, :], xT_t[:, b * 128:b * 128 + nb],
                                 w1_t[:], start=True, stop=True)
                tnm = eps.tile([128, HID], F16, tag="tnm")
                nc.vector.tensor_scalar(
                    out=tnm[:nb, :], in0=tps[:nb, :],
                    scalar1=sc1_t[:nb, b:b + 1], scalar2=None,
                    op0=mybir.AluOpType.mult)
                nc.sync.dma_start(ybounce[0][b * 128:b * 128 + nb, :], tnm[:nb, :])
            nc.gpsimd.collective_compute(
                "AllGather", mybir.AluOpType.bypass,
                replica_groups=[list(range(NCORES))],
                ins=[ybounce[0][:].opt()], outs=[yfull[0][:].opt()])

            gacc = gpsum.tile([128, GSPAN], F32)

            # ---- layers ----
            for layer in range(3):
                ytab = yfull[layer]
                wnext = [w2_t, w3_t, None][layer]
                scale_t = sc12_t if layer < 2 else sc3_t
                for w in range(NWIN):
                    gewt = []
                    cwt = []
                    for q in range(NPAGE):
                        call_id = w * NPAGE + q
                        ge = gew.tile([128, CHPC, HID], F16, tag="ge")
                        nc.gpsimd.dma_gather(
                            out_ap=ge[:],
                            in_ap=ytab[q * PAGE:(q + 1) * PAGE, :],
                            idxs_ap=idx_t[:, call_id * CCOL:(call_id + 1) * CCOL],
                            num_idxs=CALL, num_idxs_reg=CALL, elem_size=HID,
                            single_packet=False, queue_num=q)
                        gewt.append(ge)
                        ct = cw.tile([128, CHPC, 128], F16, tag="ct")
                        cbase = call_id * CHPC
                        for s0 in range(0, CHPC, 8):
                            kk = min(8, CHPC - s0)
                            nc.vector.tensor_tensor(
                                out=ct[:, s0:s0 + kk, :],
                                in0=slot_t[:, cbase + s0:cbase + s0 + kk, None]
                                    .to_broadcast([128, kk, 128]),
                                in1=iota_t[:, :kk, :],
                                op=mybir.AluOpType.is_equal)
                        cwt.append(ct)
                    sps = spsum.tile([128, WINB * 128], F32)
                    for cell in range(WINB):
                        for q in range(NPAGE):
                            for j in range(BUDGET):
                                ch = cell * BUDGET + j
                                nc.tensor.matmul(
                                    sps[:, cell * 128:(cell + 1) * 128],
                                    cwt[q][:, ch, :], gewt[q][:, ch, :],
                                    start=(q == 0 and j == 0),
                                    stop=(q == NPAGE - 1 and j == BUDGET - 1))
                    if layer == 2:
                        nwb = min(WINB, NBLK - w * WINB)
                        pw = pwin.tile([128, WINB, GSPAN], F16)
                        nc.sync.dma_start(
                            pw[:, :nwb, :],
                            pstr[:, w * WINB * GSPAN:(w * WINB + nwb) * GSPAN]
                            .rearrange("p (a g) -> p a g", a=nwb))
                    for cell in range(WINB):
                        b = w * WINB + cell
                        if b >= NBLK:
                            continue
                        nb = 128 if b < NBLK - 1 else LASTN
                        ynm = eps.tile([128, HID], F16, tag="ynm")
                        nc.vector.tensor_scalar(
                            out=ynm[:, :], in0=sps[:, cell * 128:(cell + 1) * 128],
                            scalar1=scale_t[:, b:b + 1], scalar2=0.0,
                            op0=mybir.AluOpType.mult, op1=mybir.AluOpType.max)
                        if layer < 2:
                            trp = trpsum.tile([128, HID], F16)
                            nc.tensor.transpose(trp[:], ynm[:], identh[:])
                            ytf = eps.tile([128, HID], F16, tag="ytf")
                            nc.vector.tensor_copy(ytf[:], trp[:])
                            tps = tpsum.tile([128, HID], F32)
                            nc.tensor.matmul(tps[:nb, :], ytf[:, :nb], wnext[:],
                                             start=True, stop=True)
                            tnm = eps.tile([128, HID], F16, tag="tnm")
                            nc.scalar.copy(tnm[:nb, :], tps[:nb, :])
                            nc.sync.dma_start(
                                ybounce[layer + 1][b * 128:b * 128 + nb, :],
                                tnm[:nb, :])
                        else:
                            nc.tensor.matmul(
                                gacc[:], ynm[:nb, :], pw[:nb, cell, :],
                                start=(b == 0), stop=(b == NBLK - 1))
                if layer < 2:
                    nc.gpsimd.collective_compute(
                        "AllGather", mybir.AluOpType.bypass,
                        replica_groups=[list(range(NCORES))],
                        ins=[ybounce[layer + 1][:].opt()],
                        outs=[yfull[layer + 1][:].opt()])

            # ---- classifier on pooled partials ----
            gt = eps.tile([128, GSPAN], F16, tag="gt")
            nc.vector.tensor_copy(gt[:], gacc[:])
            zps = gpsum.tile([N_CLS, GSPAN], F32, tag="zps")
            nc.tensor.matmul(zps[:], wc_t[:], gt[:], start=True, stop=True)
            zsb = eps.tile([N_CLS, GSPAN], F32, tag="zsb")
            nc.vector.tensor_copy(zsb[:], zps[:])
            nc.sync.dma_start(zc[:], zsb[:])
    nc.compile()
    return nc


def kernel(**inputs):
    global _COMPILED
    per_core, shared, cnt = _host_prep(**inputs)
    if _COMPILED is None:
        _COMPILED = _build_nc()
    nc = _COMPILED
    in_maps = []
    for c in range(NCORES):
        pc = per_core[c]
        in_maps.append({
            "xT": pc["xT"], "idxs": pc["idxs"], "slots": pc["slots"],
            "iota8": shared["iota8"],
            "W1f": shared["W1f"], "W2f": shared["W2f"], "W3f": shared["W3f"],
            "Wcf": shared["Wcf"],
            "sc1": pc["sc1"], "sc12": pc["sc12"], "sc3": pc["sc3"],
            "pstr": pc["pstr"],
        })
    res = run_bass_kernel_spmd(nc, in_maps, core_ids=list(range(NCORES)))
    Z = np.zeros((N_GRAPHS, N_CLS), np.float64)
    for c in range(NCORES):
        zc_c = res.results[c]["zc"].astype(np.float64)  # [16, GSPAN]
        g0, span = per_core[c]["g0"], per_core[c]["span"]
        Z[g0:g0 + span] += zc_c[:, :span].T
    Z = Z + np.asarray(inputs["bc"], np.float64)[None, :]
    return Z.astype(np.float32)


# revision 8
# speedup vs baseline: 10.4932x; 10.4932x over previous
"""GNN message-passing (3x GraphConv + mean-pool + classifier) on 8 Trainium2 cores.

Strategy (self-contained, hardcoded for the nn_Classifier_74019466379909 shapes):
  - Nodes dst-sharded 8 ways (12500/core). Edges assigned to dst-owner core.
  - Per layer: T = (ns*h) @ W computed on own nodes, AllGather -> full Y table
    (fp16, node-major rows); aggregation gathers Y[src] rows via dma_gather
    (4 SWDGE queues, int16 page-local indices over 4x25000-row pages) and
    segment-sums them on the TensorEngine with on-device-built 0/1 one-hot
    matrices (DVE iota-compare). Norms are folded: norm_src into the next
    transform, norm_dst (+relu) into a single fused DVE op per node block.
  - Graph mean-pool via a host-built per-node (1/cnt) one-hot P stream on PE;
    tiny per-core partial logits are summed on the host (boundary graphs
    overlap two cores).
"""
import sys
import numpy as np

sys.path.insert(0, "/opt/trn_rl_repo")

import concourse.bass as bass  # noqa: E402
import concourse.bacc as bacc  # noqa: E402
import concourse.mybir as mybir  # noqa: E402
import concourse.tile as tile  # noqa: E402
from concourse.masks import make_identity  # noqa: E402
from concourse.bass_utils import run_bass_kernel_spmd  # noqa: E402

# problem constants
N_NODES = 100000
N_EDGES = 1600000
N_GRAPHS = 1000
IN_DIM, HID, N_CLS = 95, 128, 16

NCORES = 8
B = N_NODES // NCORES            # 12500 nodes per core
NBLK = (B + 127) // 128          # 98 blocks (last = 84 rows)
LASTN = B - 127 * (NBLK - 1) - (128 - 128)  # rows in last block
LASTN = B - (NBLK - 1) * 128     # 84
WINB = 4                         # blocks per window
NWIN = (NBLK + WINB - 1) // WINB  # 25 windows (last has 2 real blocks)
NPAGE = 4
PAGE = N_NODES // NPAGE          # 25000 rows per gather page
BUDGET = 5                       # chunks (of 128 idx) per (block x page) cell
CELL = BUDGET * 128              # 640 idx slots per cell
CALL = WINB * CELL               # 2560 idxs per dma_gather call
CCOL = CALL // 16                # 160 int16 cols per call in idx stream
NCALL = NWIN * NPAGE             # 100 calls per layer
CHPC = WINB * BUDGET             # 20 chunks per call
NCHUNK = NCALL * CHPC            # 2000 chunks per layer
GSPAN = 192                      # padded per-core graph span for pooling

F16 = mybir.dt.float16
F32 = mybir.dt.float32
I16 = mybir.dt.int16

_COMPILED = None  # (nc,) cache across calls in one process


def _host_prep(x, src, dst, graph_id, W1, b1, W2, b2, W3, b3, Wc, bc):
    """Build all per-core input streams. Index math only (+ dtype marshaling)."""
    src = np.asarray(src).astype(np.int64)
    dst = np.asarray(dst).astype(np.int64)
    graph_id = np.asarray(graph_id).astype(np.int64)
    x = np.asarray(x, dtype=np.float32)
    assert np.all(np.asarray(b1) == 0) and np.all(np.asarray(b2) == 0) and np.all(
        np.asarray(b3) == 0
    ), "kernel assumes zero conv biases (relu/scale folding)"

    deg_out = np.clip(np.bincount(src, minlength=N_NODES), 1, None).astype(np.float64)
    deg_in = np.clip(np.bincount(dst, minlength=N_NODES), 1, None).astype(np.float64)
    ns = (deg_out ** -0.5).astype(np.float32)
    nd = (deg_in ** -0.5).astype(np.float32)
    cnt = np.clip(np.bincount(graph_id, minlength=N_GRAPHS), 1, None).astype(np.float32)

    core_of = dst // B
    per_core = []
    for c in range(NCORES):
        m = core_of == c
        es = src[m]
        ed = dst[m] - c * B
        blk = ed >> 7
        page = es // PAGE
        lrow = (es - page * PAGE).astype(np.int64)
        slot = (ed & 127).astype(np.int64)

        idx_stream = np.zeros((NCALL, CALL), np.int64)  # page-local row per slot
        slot_stream = np.full((NCALL, CALL), -1.0, np.float32)

        # bucket edges by (window, page, block-in-window)
        order = np.lexsort((es, page, blk))
        es_o, blk_o, page_o, lrow_o, slot_o = (
            es[order], blk[order], page[order], lrow[order], slot[order])
        cell_key = blk_o * NPAGE + page_o
        starts = np.searchsorted(cell_key, np.arange(NBLK * NPAGE + 1))
        for b in range(NBLK):
            w, cw = divmod(b, WINB)
            for q in range(NPAGE):
                lo, hi = starts[b * NPAGE + q], starts[b * NPAGE + q + 1]
                n = hi - lo
                assert n <= CELL, f"cell overflow core {c} blk {b} page {q}: {n}"
                call_id = w * NPAGE + q
                base = cw * CELL
                idx_stream[call_id, base:base + n] = lrow_o[lo:hi]
                slot_stream[call_id, base:base + n] = slot_o[lo:hi]
                if n < CELL:  # mid-call pads: repeat a valid row (slot=-1)
                    fill = lrow_o[hi - 1] if n > 0 else 0
                    idx_stream[call_id, base + n:base + CELL] = fill

        # wrap idxs: flat position i -> [i%16, i//16], replicated to 128 partitions
        idx16 = idx_stream.reshape(NCALL, CCOL, 16).transpose(0, 2, 1)  # [call,16,160]
        idx128 = np.tile(idx16, (1, 8, 1)).transpose(1, 0, 2).reshape(128, NCALL * CCOL)
        # slot stream: chunk j covers positions [128j,128j+128); [128, NCHUNK]
        slot128 = slot_stream.reshape(NCALL * CHPC, 128).T.copy()

        # pooling stream
        gid_c = graph_id[c * B:(c + 1) * B]
        g0 = int(gid_c[0])
        span = int(gid_c[-1]) - g0 + 1
        assert span <= GSPAN, f"graph span {span} > {GSPAN}"
        pstream = np.zeros((128, NBLK * GSPAN), np.float32)
        for b in range(NBLK):
            nb = 128 if b < NBLK - 1 else LASTN
            rows = gid_c[b * 128:b * 128 + nb] - g0
            pstream[np.arange(nb), b * GSPAN + rows] = 1.0 / cnt[gid_c[b * 128:b * 128 + nb]]

        # per-node scalar streams [128, NBLK]
        def colify(v):
            out = np.zeros((128, NBLK), np.float32)
            flat = v[c * B:(c + 1) * B]
            out[:, :NBLK - 1] = flat[:(NBLK - 1) * 128].reshape(NBLK - 1, 128).T
            out[:LASTN, NBLK - 1] = flat[(NBLK - 1) * 128:]
            return out

        xT = np.ascontiguousarray(x[c * B:(c + 1) * B].T).astype(np.float16)  # [95,B]

        per_core.append({
            "xT": xT,
            "idxs": idx128.astype(np.int16),
            "slots": slot128.astype(np.float16),
            "sc1": colify(ns),
            "sc12": colify(ns * nd),
            "sc3": colify(nd),
            "pstr": pstream.astype(np.float16),
            "g0": g0, "span": span,
        })

    iota = np.ascontiguousarray(np.broadcast_to(
        np.arange(128, dtype=np.float16)[None, None, :], (128, 20, 128)))
    shared = {
        "iota8": iota.reshape(128, 20 * 128),
        "W1f": np.asarray(W1, np.float32).astype(np.float16),
        "W2f": np.asarray(W2, np.float32).astype(np.float16),
        "W3f": np.asarray(W3, np.float32).astype(np.float16),
        "Wcf": np.asarray(Wc, np.float32).astype(np.float16),
    }
    return per_core, shared, cnt


def _build_nc():
    nc = bacc.Bacc("TRN2", target_bir_lowering=False, debug=False,
                   num_devices=NCORES, num_swdge_queues=4)
    xT = nc.dram_tensor("xT", [IN_DIM, B], F16, kind="ExternalInput")
    idxs = nc.dram_tensor("idxs", [128, NCALL * CCOL], I16, kind="ExternalInput")
    slots = nc.dram_tensor("slots", [128, NCHUNK], F16, kind="ExternalInput")
    iota8 = nc.dram_tensor("iota8", [128, 20 * 128], F16, kind="ExternalInput")
    W1f = nc.dram_tensor("W1f", [IN_DIM, HID], F16, kind="ExternalInput")
    W2f = nc.dram_tensor("W2f", [HID, HID], F16, kind="ExternalInput")
    W3f = nc.dram_tensor("W3f", [HID, HID], F16, kind="ExternalInput")
    Wcf = nc.dram_tensor("Wcf", [HID, N_CLS], F16, kind="ExternalInput")
    sc1 = nc.dram_tensor("sc1", [128, NBLK], F32, kind="ExternalInput")
    sc12 = nc.dram_tensor("sc12", [128, NBLK], F32, kind="ExternalInput")
    sc3 = nc.dram_tensor("sc3", [128, NBLK], F32, kind="ExternalInput")
    pstr = nc.dram_tensor("pstr", [128, NBLK * GSPAN], F16, kind="ExternalInput")
    zc = nc.dram_tensor("zc", [N_CLS, GSPAN], F32, kind="ExternalOutput")

    with tile.TileContext(nc) as tc:
        with (
            tc.tile_pool(name="res", bufs=1) as res,
            tc.tile_pool(name="dram", bufs=1, space="DRAM") as dram,
            tc.tile_pool(name="gew", bufs=10) as gew,
            tc.tile_pool(name="cw", bufs=10) as cw,
            tc.tile_pool(name="eps", bufs=6) as eps,
            tc.tile_pool(name="pwin", bufs=2) as pwin,
            tc.tile_pool(name="spsum", bufs=2, space="PSUM") as spsum,
            tc.tile_pool(name="tpsum", bufs=2, space="PSUM") as tpsum,
            tc.tile_pool(name="trpsum", bufs=2, space="PSUM") as trpsum,
            tc.tile_pool(name="gpsum", bufs=1, space="PSUM") as gpsum,
        ):
            # resident loads
            idx_t = res.tile([128, NCALL * CCOL], I16)
            slot_t = res.tile([128, NCHUNK], F16)
            iota_t = res.tile([128, 20, 128], F16)
            xT_t = res.tile([IN_DIM, B], F16)
            w1_t = res.tile([IN_DIM, HID], F16)
            w2_t = res.tile([HID, HID], F16)
            w3_t = res.tile([HID, HID], F16)
            wc_t = res.tile([HID, N_CLS], F16)
            sc1_t = res.tile([128, NBLK], F32)
            sc12_t = res.tile([128, NBLK], F32)
            sc3_t = res.tile([128, NBLK], F32)
            ident = res.tile([128, 128], F32)
            identh = res.tile([128, 128], F16)
            nc.sync.dma_start(idx_t[:], idxs[:])
            nc.sync.dma_start(slot_t[:], slots[:])
            nc.sync.dma_start(iota_t[:], iota8[:].rearrange("p (a b) -> p a b", a=20))
            nc.sync.dma_start(xT_t[:], xT[:])
            nc.sync.dma_start(w1_t[:], W1f[:])
            nc.sync.dma_start(w2_t[:], W2f[:])
            nc.sync.dma_start(w3_t[:], W3f[:])
            nc.sync.dma_start(wc_t[:], Wcf[:])
            nc.sync.dma_start(sc1_t[:], sc1[:])
            nc.sync.dma_start(sc12_t[:], sc12[:])
            nc.sync.dma_start(sc3_t[:], sc3[:])
            make_identity(nc, ident[:])
            nc.vector.tensor_copy(identh[:], ident[:])

            ybounce = [dram.tile([B, HID], F16, name=f"yb{i}", tag=f"yb{i}") for i in range(3)]
            yfull = [dram.tile([N_NODES, HID], F16, name=f"yf{i}", tag=f"yf{i}") for i in range(3)]

            # ---- phase T1: yb1 = (x @ W1) * ns (node-major fp16) ----
            for b in range(NBLK):
                nb = 128 if b < NBLK - 1 else LASTN
                tps = tpsum.tile([128, HID], F32)
                nc.tensor.matmul(tps[:nb, :], xT_t[:, b * 128:b * 128 + nb],
                                 w1_t[:], start=True, stop=True)
                tnm = eps.tile([128, HID], F16, tag="tnm")
                nc.vector.tensor_scalar(
                    out=tnm[:nb, :], in0=tps[:nb, :],
                    scalar1=sc1_t[:nb, b:b + 1], scalar2=None,
                    op0=mybir.AluOpType.mult)
                nc.sync.dma_start(ybounce[0][b * 128:b * 128 + nb, :], tnm[:nb, :])
            nc.gpsimd.collective_compute(
                "AllGather", mybir.AluOpType.bypass,
                replica_groups=[list(range(NCORES))],
                ins=[ybounce[0][:].opt()], outs=[yfull[0][:].opt()])

            gacc = gpsum.tile([128, GSPAN], F32)

            # ---- layers ----
            for layer in range(3):
                ytab = yfull[layer]
                wnext = [w2_t, w3_t, None][layer]
                scale_t = sc12_t if layer < 2 else sc3_t
                for w in range(NWIN):
                    gewt = []
                    cwt = []
                    for q in range(NPAGE):
                        call_id = w * NPAGE + q
                        ge = gew.tile([128, CHPC, HID], F16, tag="ge")
                        nc.gpsimd.dma_gather(
                            out_ap=ge[:],
                            in_ap=ytab[q * PAGE:(q + 1) * PAGE, :],
                            idxs_ap=idx_t[:, call_id * CCOL:(call_id + 1) * CCOL],
                            num_idxs=CALL, num_idxs_reg=CALL, elem_size=HID,
                            single_packet=False, queue_num=q)
                        gewt.append(ge)
                        ct = cw.tile([128, CHPC, 128], F16, tag="ct")
                        cbase = call_id * CHPC
                        nc.vector.tensor_tensor(
                            out=ct[:],
                            in0=slot_t[:, cbase:cbase + CHPC, None]
                                .to_broadcast([128, CHPC, 128]),
                            in1=iota_t[:, :CHPC, :],
                            op=mybir.AluOpType.is_equal)
                        cwt.append(ct)
                    sps = spsum.tile([128, WINB * 128], F32)
                    for cell in range(WINB):
                        for q in range(NPAGE):
                            for j in range(BUDGET):
                                ch = cell * BUDGET + j
                                nc.tensor.matmul(
                                    sps[:, cell * 128:(cell + 1) * 128],
                                    cwt[q][:, ch, :], gewt[q][:, ch, :],
                                    start=(q == 0 and j == 0),
                                    stop=(q == NPAGE - 1 and j == BUDGET - 1))
                    if layer == 2:
                        nwb = min(WINB, NBLK - w * WINB)
                        pw = pwin.tile([128, WINB, GSPAN], F16)
                        nc.sync.dma_start(
                            pw[:, :nwb, :],
                            pstr[:, w * WINB * GSPAN:(w * WINB + nwb) * GSPAN]
                            .rearrange("p (a g) -> p a g", a=nwb))
                    for cell in range(WINB):
                        b = w * WINB + cell
                        if b >= NBLK:
                            continue
                        nb = 128 if b < NBLK - 1 else LASTN
                        ynm = eps.tile([128, HID], F16, tag="ynm")
                        nc.vector.tensor_scalar(
                            out=ynm[:, :], in0=sps[:, cell * 128:(cell + 1) * 128],
                            scalar1=scale_t[:, b:b + 1], scalar2=0.0,
                            op0=mybir.AluOpType.mult, op1=mybir.AluOpType.max)
                        if layer < 2:
                            trp = trpsum.tile([128, HID], F16)
                            nc.tensor.transpose(trp[:], ynm[:], identh[:])
                            ytf = eps.tile([128, HID], F16, tag="ytf")
                            nc.vector.tensor_copy(ytf[:], trp[:])
                            tps = tpsum.tile([128, HID], F32)
                            nc.tensor.matmul(tps[:nb, :], ytf[:, :nb], wnext[:],
                                             start=True, stop=True)
                            tnm = eps.tile([128, HID], F16, tag="tnm")
                            nc.scalar.copy(tnm[:nb, :], tps[:nb, :])
                            nc.sync.dma_start(
                                ybounce[layer + 1][b * 128:b * 128 + nb, :],
                                tnm[:nb, :])
                        else:
                            nc.tensor.matmul(
                                gacc[:], ynm[:nb, :], pw[:nb, cell, :],
                                start=(b == 0), stop=(b == NBLK - 1))
                if layer < 2:
                    nc.gpsimd.collective_compute(
                        "AllGather", mybir.AluOpType.bypass,
                        replica_groups=[list(range(NCORES))],
                        ins=[ybounce[layer + 1][:].opt()],
                        outs=[yfull[layer + 1][:].opt()])

            # ---- classifier on pooled partials ----
            gt = eps.tile([128, GSPAN], F16, tag="gt")
            nc.vector.tensor_copy(gt[:], gacc[:])
            zps = gpsum.tile([N_CLS, GSPAN], F32, tag="zps")
            nc.tensor.matmul(zps[:], wc_t[:], gt[:], start=True, stop=True)
            zsb = eps.tile([N_CLS, GSPAN], F32, tag="zsb")
            nc.vector.tensor_copy(zsb[:], zps[:])
            nc.sync.dma_start(zc[:], zsb[:])
    nc.compile()
    return nc


def kernel(**inputs):
    global _COMPILED
    per_core, shared, cnt = _host_prep(**inputs)
    if _COMPILED is None:
        _COMPILED = _build_nc()
    nc = _COMPILED
    in_maps = []
    for c in range(NCORES):
        pc = per_core[c]
        in_maps.append({
            "xT": pc["xT"], "idxs": pc["idxs"], "slots": pc["slots"],
            "iota8": shared["iota8"],
            "W1f": shared["W1f"], "W2f": shared["W2f"], "W3f": shared["W3f"],
            "Wcf": shared["Wcf"],
            "sc1": pc["sc1"], "sc12": pc["sc12"], "sc3": pc["sc3"],
            "pstr": pc["pstr"],
        })
    res = run_bass_kernel_spmd(nc, in_maps, core_ids=list(range(NCORES)))
    Z = np.zeros((N_GRAPHS, N_CLS), np.float64)
    for c in range(NCORES):
        zc_c = res.results[c]["zc"].astype(np.float64)  # [16, GSPAN]
        g0, span = per_core[c]["g0"], per_core[c]["span"]
        Z[g0:g0 + span] += zc_c[:, :span].T
    Z = Z + np.asarray(inputs["bc"], np.float64)[None, :]
    return Z.astype(np.float32)
